# revision 10
# baseline (speedup 1.0000x reference)
# DeepGEMM-style fp8 block-quantized linear for Trainium2, 8-core SPMD.
#
# reference semantics:
#   x_dq = dequant(quant_e4m3fn(x, per-token per-128-group amax/448 scales))
#   w_dq = w_fp8 * w_scale (per 128x128 block)
#   out  = (x_dq @ w_dq.T).astype(bf16)          # fp32 accumulation
#
# Strategy (per core; 2x4 [M x N] grid => M2=2048, N2=1024 per core):
#   - TRN fp8_e4m3 tops out at 240 (vs OCP e4m3fn's 448), so quantize
#     x * (112/amax) on TRN's grid: identical rounding for normals (pure
#     exponent shift); dequantize with s4 = amax/112.
#   - scales folded into fp16 matmul operands (fp16 keeps the e4m3fn
#     values and 448-max weight values exact); on-chip xbar DMA
#     transposes into [K, *] layouts.
#   - W path: raw f32 loads split between the scalar HW DMA queue and
#     gpsimd software-DGE; one tensor_tensor does cast+block-scale
#     (f32 x f32-bcast -> f16), split across vector/gpsimd.
#   - emission interleaves W chunks with x-prep for the first m-tiles so
#     no engine queue head-blocks; later m-tiles prep one tile ahead
#     inside the matmul loop. This keeps the PE fed (no HAM re-throttle).

import numpy as np
import ml_dtypes
from contextlib import ExitStack

import concourse.bass as bass
import concourse.mybir as mybir
import concourse.tile as tile
from concourse import bacc
from concourse.bass_utils import run_bass_kernel_spmd

dt = mybir.dt

M, N, K = 4096, 4096, 7168
MSH, NSH = 2, 4                     # core grid: 2 along M, 4 along N
NCORES = MSH * NSH
BLK = 128


def bcast_inner(ap, n):
    """Append a step-0 inner dim of size n (free-dim broadcast read)."""
    return bass.AP(tensor=ap.tensor, offset=ap.offset, ap=[*ap.ap, [0, n]])


def emit_kernel(ctx, tc, o_d, x_d, w_d, ws_d, *, nq_width=256):
    nc = tc.nc
    f32, f16, f8 = dt.float32, dt.float16, dt.float8e4
    bf16 = dt.bfloat16
    M2, Kd = x_d.shape
    N2, _ = w_d.shape
    KB = Kd // BLK              # k-blocks (56)
    NB = N2 // BLK              # n-blocks (8)
    MT = M2 // BLK              # m-tiles (16)
    NQ = N2 // nq_width         # psum tiles per m-tile
    KQ = 4                      # x pipeline chunks per m-tile
    KBQ = KB // KQ              # k-blocks per x chunk (14)
    KL = Kd // KQ               # 1792
    WKH = 8                     # W k-chunks per n-block
    WKL = Kd // WKH             # 896
    WKB = KB // WKH             # 7

    wtp = ctx.enter_context(tc.tile_pool(name="wt", bufs=1))
    constp = ctx.enter_context(tc.tile_pool(name="consts", bufs=1))
    wqp = ctx.enter_context(tc.tile_pool(name="wq", bufs=4))
    wqhp = ctx.enter_context(tc.tile_pool(name="wqh", bufs=4))
    xnp = ctx.enter_context(tc.tile_pool(name="xn", bufs=5))
    scp = ctx.enter_context(tc.tile_pool(name="sc", bufs=12))
    xqp = ctx.enter_context(tc.tile_pool(name="xq", bufs=3))
    xdqp = ctx.enter_context(tc.tile_pool(name="xdq", bufs=2))
    xtp = ctx.enter_context(tc.tile_pool(name="xt", bufs=9))
    obp = ctx.enter_context(tc.tile_pool(name="ob", bufs=3))
    psp = ctx.enter_context(tc.tile_pool(name="ps", bufs=4, space="PSUM"))

    # w_scale broadcast across partitions via step-0 partition DMA read
    wsb = constp.tile([128, NB * KB], f32)
    ws_flat = ws_d.rearrange("a b -> (a b)")
    ws_b = bass.AP(tensor=ws_flat.tensor, offset=ws_flat.offset,
                   ap=[[0, 128], *ws_flat.ap])
    nc.sync.dma_start(wsb[:], ws_b)

    # stage 1 target: wt_t[p, kb, n] = w[n, kb*128+p] * ws[n//128, kb]  (fp16)
    wt_t = wtp.tile([128, KB, N2], f16)

    def emit_wchunk(i):
        nb, kh = divmod(i, WKH)
        wq = wqp.tile([128, WKL], f32, tag="wq")
        src = w_d[nb * BLK:(nb + 1) * BLK, kh * WKL:(kh + 1) * WKL]
        nc.scalar.dma_start(wq[:], src)
        wqh = wqhp.tile([128, WKL], f16, tag="wqh")
        # ~1/3 of chunks on gpsimd (its TT is ~1.6x slower)
        eng = nc.gpsimd if i % 3 == 0 else nc.vector
        eng.tensor_tensor(
            out=wqh[:].rearrange("p (kb c) -> p kb c", c=BLK),
            in0=wq[:].rearrange("p (kb c) -> p kb c", c=BLK),
            in1=bcast_inner(
                wsb[:, nb * KB + kh * WKB: nb * KB + (kh + 1) * WKB], BLK),
            op=mybir.AluOpType.mult)
        nc.scalar.dma_start(
            wt_t[:, kh * WKB:(kh + 1) * WKB, nb * BLK:(nb + 1) * BLK],
            wqh[:], transpose=True)

    xt_map = {}

    def emit_prep(mt):
        # all 4 input loads first, then scale chains, then quant/dequant:
        # keeps the sync queue free of mid-chain waits and lets the next
        # prep's loads run while this prep's vector work drains.
        xns, xngs, s4s, inv4s = [], [], [], []
        for q in range(KQ):
            xn = xnp.tile([128, KL], bf16, tag="xn")
            nc.sync.dma_start(
                xn[:], x_d[mt * BLK:(mt + 1) * BLK, q * KL:(q + 1) * KL])
            xns.append(xn)
            xngs.append(xn[:].rearrange("p (kb c) -> p kb c", c=BLK))
        for q in range(KQ):
            amax = scp.tile([128, KBQ], f32, tag="amax")
            nc.vector.reduce_max(
                amax[:], xngs[q], axis=mybir.AxisListType.X,
                apply_absolute_value=True)
            # s4 ~= max(amax, 1e-12)/112 (== 4x reference scale up to 1 ulp)
            s4 = scp.tile([128, KBQ], f32, tag="s4")
            nc.gpsimd.tensor_scalar(
                out=s4[:], in0=amax[:],
                scalar1=1e-12, scalar2=float(np.float32(1.0 / 112.0)),
                op0=mybir.AluOpType.max, op1=mybir.AluOpType.mult)
            inv4 = scp.tile([128, KBQ], f32, tag="inv4")
            nc.vector.reciprocal(inv4[:], s4[:])
            s4s.append(s4)
            inv4s.append(inv4)
        xts = []
        for q in range(KQ):
            xq = xqp.tile([128, KL], f8, tag="xq")
            xqg = xq[:].rearrange("p (kb c) -> p kb c", c=BLK)
            nc.gpsimd.tensor_tensor(
                out=xqg, in0=xngs[q], in1=bcast_inner(inv4s[q][:], BLK),
                op=mybir.AluOpType.mult)
            xdq = xdqp.tile([128, KL], f16, tag="xdq")
            xdqg = xdq[:].rearrange("p (kb c) -> p kb c", c=BLK)
            nc.vector.tensor_tensor(
                out=xdqg, in0=xqg, in1=bcast_inner(s4s[q][:], BLK),
                op=mybir.AluOpType.mult)

            xt_t = xtp.tile([128, KBQ, 128], f16, tag="xt")
            nc.scalar.dma_start(xt_t[:], xdq[:], transpose=True)
            xts.append(xt_t)
        xt_map[mt] = xts

    ob_map = {}

    def emit_mm_part(mt, nq):
        # one psum quarter of m-tile mt; must be emitted after the W chunks
        # covering columns [nq*nq_width, (nq+1)*nq_width) (Tile derives deps
        # from emission order).
        xts = xt_map[mt]
        if mt not in ob_map:
            ob = obp.tile([128, N2], bf16, tag="ob", name=f"ob{mt}")
            ob_map[mt] = ob
        ob = ob_map[mt]
        pst = psp.tile([128, nq_width], f32, tag=f"ps{nq % 2}")
        for kb in range(KB):
            nc.tensor.matmul(
                pst[:],
                xts[kb // KBQ][:, kb % KBQ, :],
                wt_t[:, kb, nq * nq_width:(nq + 1) * nq_width],
                start=(kb == 0), stop=(kb == KB - 1))
        nc.scalar.copy(ob[:, nq * nq_width:(nq + 1) * nq_width], pst[:])
        if nq == NQ - 1:
            nc.scalar.dma_start(o_d[mt * BLK:(mt + 1) * BLK, :], ob[:])

    def emit_mm(mt):
        for nq in range(NQ):
            emit_mm_part(mt, nq)

    # --- emission schedule ---
    # W chunks, x-prep, and matmul quarters are interleaved so (a) every
    # matmul is emitted after the W chunks it reads, (b) the first m-tiles'
    # quarters start as soon as their W columns land, (c) preps stay ~2
    # m-tiles ahead and no engine queue head-blocks on the W tail.
    for i in range(16):
        emit_wchunk(i)          # nb0,1  -> cols 0:256
    emit_prep(0)
    emit_prep(1)
    emit_mm_part(0, 0)
    for i in range(16, 32):
        emit_wchunk(i)          # nb2,3  -> cols 256:512
    emit_prep(2)
    emit_mm_part(0, 1)
    emit_mm_part(1, 0)
    for i in range(32, 48):
        emit_wchunk(i)          # nb4,5  -> cols 512:768
    emit_prep(3)
    emit_mm_part(0, 2)
    emit_mm_part(1, 1)
    for i in range(48, 64):
        emit_wchunk(i)          # nb6,7  -> cols 768:1024
    emit_mm_part(0, 3)
    emit_mm_part(1, 2)
    emit_mm_part(1, 3)
    for mt in range(2, MT):
        emit_mm(mt)
        if mt + 2 < MT:
            emit_prep(mt + 2)


def build_nc(m2, n2, k, **kw):
    nc = bacc.Bacc("TRN2", target_bir_lowering=False, debug=False, num_devices=NCORES)
    x_d = nc.dram_tensor("x", [m2, k], dt.bfloat16, kind="ExternalInput").ap()
    w_d = nc.dram_tensor("w", [n2, k], dt.float32, kind="ExternalInput").ap()
    ws_d = nc.dram_tensor("ws", [n2 // BLK, k // BLK], dt.float32, kind="ExternalInput").ap()
    o_d = nc.dram_tensor("o", [m2, n2], dt.bfloat16, kind="ExternalOutput").ap()
    with tile.TileContext(nc) as tc, ExitStack() as ctx:
        emit_kernel(ctx, tc, o_d, x_d, w_d, ws_d, **kw)
    nc.compile()
    return nc


_cache = {}


def _get_nc():
    if "nc" not in _cache:
        _cache["nc"] = build_nc(M // MSH, N // NSH, K)
    return _cache["nc"]


def kernel(input, weight_fp8, weight_scale, _trace=False, _trace_kwargs=None):
    input = np.asarray(input)
    if input.dtype != ml_dtypes.bfloat16:
        input = input.astype(ml_dtypes.bfloat16)
    weight_fp8 = np.asarray(weight_fp8, dtype=np.float32)
    weight_scale = np.asarray(weight_scale, dtype=np.float32)
    M2, N2 = M // MSH, N // NSH
    NSB = N2 // BLK

    in_maps = []
    for c in range(NCORES):
        mi, ni = divmod(c, NSH)
        in_maps.append({
            "x": np.ascontiguousarray(input[mi * M2:(mi + 1) * M2]),
            "w": np.ascontiguousarray(weight_fp8[ni * N2:(ni + 1) * N2]),
            "ws": np.ascontiguousarray(weight_scale[ni * NSB:(ni + 1) * NSB]),
        })

    nc = _get_nc()
    kw = {}
    if _trace:
        kw = dict(trace=True, **(_trace_kwargs or {}))
    res = run_bass_kernel_spmd(nc, in_maps, core_ids=list(range(NCORES)), **kw)

    out = np.empty((M, N), dtype=ml_dtypes.bfloat16)
    for c in range(NCORES):
        mi, ni = divmod(c, NSH)
        out[mi * M2:(mi + 1) * M2, ni * N2:(ni + 1) * N2] = res.results[c]["o"]
    if _trace:
        return out, res
    return out


# revision 12
# speedup vs baseline: 1.1433x; 1.1433x over previous
# DeepGEMM-style fp8 block-quantized linear for Trainium2, 8-core SPMD.
#
# reference semantics:
#   x_dq = dequant(quant_e4m3fn(x, per-token per-128-group amax/448 scales))
#   w_dq = w_fp8 * w_scale (per 128x128 block)
#   out  = (x_dq @ w_dq.T).astype(bf16)          # fp32 accumulation
#
# Strategy (per core; 2x4 [M x N] grid => M2=2048, N2=1024 per core):
#   - TRN fp8_e4m3 tops out at 240 (vs OCP e4m3fn's 448), so quantize
#     x * (112/amax) on TRN's grid: identical rounding for normals (pure
#     exponent shift); dequantize with s4 = amax/112.
#   - scales folded into fp16 matmul operands (fp16 keeps the e4m3fn
#     values and 448-max weight values exact, and halves bf16's rounding
#     noise); on-chip xbar DMA transposes into [K, *] layouts.
#   - quant/dequant/w-dequant are single big tensor_tensor ops using
#     step-0 (free-dim broadcast) access patterns for the per-128-group
#     scales; dequant runs on GpSimd to keep DVE under the PE roofline.
#   - psum [128, 256] quarter tiles accumulate over the 56 k-blocks;
#     n-quarter ordering lets matmuls start while W is still streaming in.

import numpy as np
import ml_dtypes
from contextlib import ExitStack

import concourse.bass as bass
import concourse.mybir as mybir
import concourse.tile as tile
from concourse import bacc
from concourse.bass_utils import run_bass_kernel_spmd

dt = mybir.dt

M, N, K = 4096, 4096, 7168
MSH, NSH = 2, 4                     # core grid: 2 along M, 4 along N
NCORES = MSH * NSH
BLK = 128


def bcast_inner(ap, n):
    """Append a step-0 inner dim of size n (free-dim broadcast read)."""
    return bass.AP(tensor=ap.tensor, offset=ap.offset, ap=[*ap.ap, [0, n]])


def emit_kernel(ctx, tc, o_d, x_d, w_d, ws_d, *, xq_engine="gpsimd", xdq_engine="vector", sc_engine="gpsimd", nq_width=256):
    nc = tc.nc
    f32, f16, f8 = dt.float32, dt.float16, dt.float8e4
    bf16 = dt.bfloat16
    M2, Kd = x_d.shape
    N2, _ = w_d.shape
    KB = Kd // BLK              # k-blocks
    NB = N2 // BLK              # n-blocks
    MT = M2 // BLK              # m-tiles
    NQ = N2 // nq_width         # psum tiles per m-tile
    KQ = 4                      # x pipeline chunks per m-tile
    KBQ = KB // KQ
    KHW = 2                     # w cast/transpose k-chunks per n-block
    assert KB % KQ == 0 and NB % 2 == 0

    wtp = ctx.enter_context(tc.tile_pool(name="wt", bufs=1))
    constp = ctx.enter_context(tc.tile_pool(name="consts", bufs=1))
    wqp = ctx.enter_context(tc.tile_pool(name="wq", bufs=2))
    xnp = ctx.enter_context(tc.tile_pool(name="xn", bufs=6))
    scp = ctx.enter_context(tc.tile_pool(name="sc", bufs=8))
    xqp = ctx.enter_context(tc.tile_pool(name="xq", bufs=3))
    xdqp = ctx.enter_context(tc.tile_pool(name="xdq", bufs=4))
    xtp = ctx.enter_context(tc.tile_pool(name="xt", bufs=9))
    obp = ctx.enter_context(tc.tile_pool(name="ob", bufs=2))
    psp = ctx.enter_context(tc.tile_pool(name="ps", bufs=2, space="PSUM"))

    # w_scale broadcast across partitions via step-0 partition DMA read
    wsb = constp.tile([128, NB * KB], f32)
    ws_flat = ws_d.rearrange("a b -> (a b)")
    ws_b = bass.AP(tensor=ws_flat.tensor, offset=ws_flat.offset,
                   ap=[[0, 128], *ws_flat.ap])
    nc.gpsimd.dma_start(wsb[:], ws_b)

    # stage 1: W -> wt_t[p, kb, n] = w[n, kb*128+p] * ws[n//128, kb]  (fp16)
    wt_t = wtp.tile([128, KB, N2], f16)
    KHL = Kd // KHW
    KBH = KB // KHW
    for nb in range(NB):
        for kh in range(KHW):
            wq = wqp.tile([128, KHL], f16, tag="wq")
            nc.gpsimd.dma_start(
                wq[:], w_d[nb * BLK:(nb + 1) * BLK, kh * KHL:(kh + 1) * KHL])
            nc.sync.dma_start(
                wt_t[:, kh * KBH:(kh + 1) * KBH, nb * BLK:(nb + 1) * BLK],
                wq[:], transpose=True)
            sl = wt_t[:, kh * KBH:(kh + 1) * KBH, nb * BLK:(nb + 1) * BLK]
            nc.vector.tensor_tensor(
                out=sl, in0=sl,
                in1=bcast_inner(
                    wsb[:, nb * KB + kh * KBH: nb * KB + (kh + 1) * KBH], BLK),
                op=mybir.AluOpType.mult)

    # stage 2: per m-tile quant + matmul
    xq_eng = getattr(nc, xq_engine)
    xdq_eng = getattr(nc, xdq_engine)
    sc_eng = getattr(nc, sc_engine)
    KL = Kd // KQ
    for mt in range(MT):
        xt_qs = []
        for q in range(KQ):
            xn = xnp.tile([128, KL], bf16, tag="xn")
            nc.sync.dma_start(xn[:], x_d[mt * BLK:(mt + 1) * BLK, q * KL:(q + 1) * KL])
            xng = xn[:].rearrange("p (kb c) -> p kb c", c=BLK)

            amax = scp.tile([128, KBQ], f32, tag="amax")
            nc.vector.reduce_max(
                amax[:], xng, axis=mybir.AxisListType.X, apply_absolute_value=True)
            # s4 ~= max(amax, 1e-12)/112 (== 4x reference scale up to 1 ulp)
            s4 = scp.tile([128, KBQ], f32, tag="s4")
            sc_eng.tensor_scalar(
                out=s4[:], in0=amax[:],
                scalar1=1e-12, scalar2=float(np.float32(1.0 / 112.0)),
                op0=mybir.AluOpType.max, op1=mybir.AluOpType.mult)
            inv4 = scp.tile([128, KBQ], f32, tag="inv4")
            nc.vector.reciprocal(inv4[:], s4[:])

            xq = xqp.tile([128, KL], f8, tag="xq")
            xqg = xq[:].rearrange("p (kb c) -> p kb c", c=BLK)
            xq_eng.tensor_tensor(
                out=xqg, in0=xng, in1=bcast_inner(inv4[:], BLK),
                op=mybir.AluOpType.mult)
            xdq = xdqp.tile([128, KL], f16, tag="xdq")
            xdqg = xdq[:].rearrange("p (kb c) -> p kb c", c=BLK)
            xdq_eng.tensor_tensor(
                out=xdqg, in0=xqg, in1=bcast_inner(s4[:], BLK),
                op=mybir.AluOpType.mult)

            xt_t = xtp.tile([128, KBQ, 128], f16, tag="xt")
            nc.sync.dma_start(xt_t[:], xdq[:], transpose=True)
            xt_qs.append(xt_t)

        ob = obp.tile([128, N2], bf16, tag="ob")
        for nq in range(NQ):
            pst = psp.tile([128, nq_width], f32, tag=f"ps{nq}")
            for kb in range(KB):
                nc.tensor.matmul(
                    pst[:],
                    xt_qs[kb // KBQ][:, kb % KBQ, :],
                    wt_t[:, kb, nq * nq_width:(nq + 1) * nq_width],
                    start=(kb == 0), stop=(kb == KB - 1))
            nc.scalar.copy(ob[:, nq * nq_width:(nq + 1) * nq_width], pst[:])
        nc.sync.dma_start(o_d[mt * BLK:(mt + 1) * BLK, :], ob[:])


def build_nc(m2, n2, k, **kw):
    nc = bacc.Bacc("TRN2", target_bir_lowering=False, debug=False, num_devices=NCORES)
    x_d = nc.dram_tensor("x", [m2, k], dt.bfloat16, kind="ExternalInput").ap()
    w_d = nc.dram_tensor("w", [n2, k], dt.float32, kind="ExternalInput").ap()
    ws_d = nc.dram_tensor("ws", [n2 // BLK, k // BLK], dt.float32, kind="ExternalInput").ap()
    o_d = nc.dram_tensor("o", [m2, n2], dt.bfloat16, kind="ExternalOutput").ap()
    with tile.TileContext(nc) as tc, ExitStack() as ctx:
        emit_kernel(ctx, tc, o_d, x_d, w_d, ws_d, **kw)
    nc.compile()
    return nc


_cache = {}


def _get_nc():
    if "nc" not in _cache:
        _cache["nc"] = build_nc(M // MSH, N // NSH, K)
    return _cache["nc"]


def kernel(input, weight_fp8, weight_scale, _trace=False, _trace_kwargs=None):
    input = np.asarray(input)
    if input.dtype != ml_dtypes.bfloat16:
        input = input.astype(ml_dtypes.bfloat16)
    weight_fp8 = np.asarray(weight_fp8, dtype=np.float32)
    weight_scale = np.asarray(weight_scale, dtype=np.float32)
    M2, N2 = M // MSH, N // NSH
    NSB = N2 // BLK

    in_maps = []
    for c in range(NCORES):
        mi, ni = divmod(c, NSH)
        in_maps.append({
            "x": np.ascontiguousarray(input[mi * M2:(mi + 1) * M2]),
            "w": np.ascontiguousarray(weight_fp8[ni * N2:(ni + 1) * N2]),
            "ws": np.ascontiguousarray(weight_scale[ni * NSB:(ni + 1) * NSB]),
        })

    nc = _get_nc()
    kw = {}
    if _trace:
        kw = dict(trace=True, **(_trace_kwargs or {}))
    res = run_bass_kernel_spmd(nc, in_maps, core_ids=list(range(NCORES)), **kw)

    out = np.empty((M, N), dtype=ml_dtypes.bfloat16)
    for c in range(NCORES):
        mi, ni = divmod(c, NSH)
        out[mi * M2:(mi + 1) * M2, ni * N2:(ni + 1) * N2] = res.results[c]["o"]
    if _trace:
        return out, res
    return out



# revision 15
# speedup vs baseline: 1.1992x; 1.0489x over previous
# DeepGEMM-style fp8 block-quantized linear for Trainium2, 8-core SPMD.
#
# reference semantics:
#   x_dq = dequant(quant_e4m3fn(x, per-token per-128-group amax/448 scales))
#   w_dq = w_fp8 * w_scale (per 128x128 block)
#   out  = (x_dq @ w_dq.T).astype(bf16)          # fp32 accumulation
#
# Strategy (per core; 2x4 [M x N] grid => M2=2048, N2=1024 per core):
#   - TRN fp8_e4m3 tops out at 240 (vs OCP e4m3fn's 448), so quantize
#     x * (112/amax) on TRN's grid: identical rounding for normals (pure
#     exponent shift); dequantize with s4 = amax/112.
#   - scales folded into fp16 matmul operands (fp16 keeps the e4m3fn
#     values and 448-max weight values exact, and halves bf16's rounding
#     noise); on-chip xbar DMA transposes into [K, *] layouts.
#   - quant/dequant/w-dequant are single big tensor_tensor ops using
#     step-0 (free-dim broadcast) access patterns for the per-128-group
#     scales; dequant runs on GpSimd to keep DVE under the PE roofline.
#   - psum [128, 256] quarter tiles accumulate over the 56 k-blocks;
#     n-quarter ordering lets matmuls start while W is still streaming in.

import numpy as np
import ml_dtypes
from contextlib import ExitStack

import concourse.bass as bass
import concourse.mybir as mybir
import concourse.tile as tile
from concourse import bacc
from concourse.bass_utils import run_bass_kernel_spmd

dt = mybir.dt

M, N, K = 4096, 4096, 7168
MSH, NSH = 2, 4                     # core grid: 2 along M, 4 along N
NCORES = MSH * NSH
BLK = 128


def bcast_inner(ap, n):
    """Append a step-0 inner dim of size n (free-dim broadcast read)."""
    return bass.AP(tensor=ap.tensor, offset=ap.offset, ap=[*ap.ap, [0, n]])


def emit_kernel(ctx, tc, o_d, x_d, w_d, ws_d, *, xq_engine="gpsimd", xdq_engine="vector", sc_engine="gpsimd", nq_width=256):
    nc = tc.nc
    f32, f16, f8 = dt.float32, dt.float16, dt.float8e4
    bf16 = dt.bfloat16
    M2, Kd = x_d.shape
    N2, _ = w_d.shape
    KB = Kd // BLK              # k-blocks
    NB = N2 // BLK              # n-blocks
    MT = M2 // BLK              # m-tiles
    NQ = N2 // nq_width         # psum tiles per m-tile
    KQ = 4                      # x pipeline chunks per m-tile
    KBQ = KB // KQ
    KHW = 2                     # w cast/transpose k-chunks per n-block
    assert KB % KQ == 0 and NB % 2 == 0

    wtp = ctx.enter_context(tc.tile_pool(name="wt", bufs=1))
    constp = ctx.enter_context(tc.tile_pool(name="consts", bufs=1))
    wqp = ctx.enter_context(tc.tile_pool(name="wq", bufs=3))
    wq32p = ctx.enter_context(tc.tile_pool(name="wq32", bufs=2))
    xnp = ctx.enter_context(tc.tile_pool(name="xn", bufs=4))
    scp = ctx.enter_context(tc.tile_pool(name="sc", bufs=8))
    xqp = ctx.enter_context(tc.tile_pool(name="xq", bufs=3))
    xdqp = ctx.enter_context(tc.tile_pool(name="xdq", bufs=3))
    xtp = ctx.enter_context(tc.tile_pool(name="xt", bufs=9))
    obp = ctx.enter_context(tc.tile_pool(name="ob", bufs=2))
    psp = ctx.enter_context(tc.tile_pool(name="ps", bufs=2, space="PSUM"))

    # w_scale broadcast across partitions via step-0 partition DMA read
    wsb = constp.tile([128, NB * KB], f32)
    ws_flat = ws_d.rearrange("a b -> (a b)")
    ws_b = bass.AP(tensor=ws_flat.tensor, offset=ws_flat.offset,
                   ap=[[0, 128], *ws_flat.ap])
    nc.gpsimd.dma_start(wsb[:], ws_b)

    # stage 1: W -> wt_t[p, kb, n] = w[n, kb*128+p] * ws[n//128, kb]  (fp16)
    # quarter-chunks; loads split across the gpsimd software-DGE (casting)
    # and the scalar HW queue (raw f32 + ACT cast) so W streams in ~2x
    # faster than the baseline single-DGE path; block-scale applied
    # in-place post-transpose, split vector/gpsimd.
    wt_t = wtp.tile([128, KB, N2], f16)
    WKH = 4
    WKL = Kd // WKH             # 1792
    WKB = KB // WKH             # 14
    for i in range(NB * WKH):
        nb, kh = divmod(i, WKH)
        src_ap = w_d[nb * BLK:(nb + 1) * BLK, kh * WKL:(kh + 1) * WKL]
        wq = wqp.tile([128, WKL], f16, tag="wq")
        if i % 2 == 0:
            nc.gpsimd.dma_start(wq[:], src_ap)
        else:
            wq32 = wq32p.tile([128, WKL], f32, tag="wq32")
            nc.scalar.dma_start(wq32[:], src_ap)
            nc.scalar.activation(wq[:], wq32[:],
                                 mybir.ActivationFunctionType.Copy)
        nc.sync.dma_start(
            wt_t[:, kh * WKB:(kh + 1) * WKB, nb * BLK:(nb + 1) * BLK],
            wq[:], transpose=True)
        sl = wt_t[:, kh * WKB:(kh + 1) * WKB, nb * BLK:(nb + 1) * BLK]
        eng = nc.gpsimd if i % 3 == 0 else nc.vector
        eng.tensor_tensor(
            out=sl, in0=sl,
            in1=bcast_inner(
                wsb[:, nb * KB + kh * WKB: nb * KB + (kh + 1) * WKB], BLK),
            op=mybir.AluOpType.mult)

    # stage 2: per m-tile quant + matmul
    xq_eng = getattr(nc, xq_engine)
    xdq_eng = getattr(nc, xdq_engine)
    sc_eng = getattr(nc, sc_engine)
    KL = Kd // KQ
    for mt in range(MT):
        xt_qs = []
        for q in range(KQ):
            xn = xnp.tile([128, KL], bf16, tag="xn")
            nc.sync.dma_start(xn[:], x_d[mt * BLK:(mt + 1) * BLK, q * KL:(q + 1) * KL])
            xng = xn[:].rearrange("p (kb c) -> p kb c", c=BLK)

            amax = scp.tile([128, KBQ], f32, tag="amax")
            nc.vector.reduce_max(
                amax[:], xng, axis=mybir.AxisListType.X, apply_absolute_value=True)
            # s4 ~= max(amax, 1e-12)/112 (== 4x reference scale up to 1 ulp)
            s4 = scp.tile([128, KBQ], f32, tag="s4")
            sc_eng.tensor_scalar(
                out=s4[:], in0=amax[:],
                scalar1=1e-12, scalar2=float(np.float32(1.0 / 112.0)),
                op0=mybir.AluOpType.max, op1=mybir.AluOpType.mult)
            inv4 = scp.tile([128, KBQ], f32, tag="inv4")
            nc.vector.reciprocal(inv4[:], s4[:])

            xq = xqp.tile([128, KL], f8, tag="xq")
            xqg = xq[:].rearrange("p (kb c) -> p kb c", c=BLK)
            xq_eng.tensor_tensor(
                out=xqg, in0=xng, in1=bcast_inner(inv4[:], BLK),
                op=mybir.AluOpType.mult)
            xdq = xdqp.tile([128, KL], f16, tag="xdq")
            xdqg = xdq[:].rearrange("p (kb c) -> p kb c", c=BLK)
            xdq_eng.tensor_tensor(
                out=xdqg, in0=xqg, in1=bcast_inner(s4[:], BLK),
                op=mybir.AluOpType.mult)

            xt_t = xtp.tile([128, KBQ, 128], f16, tag="xt")
            nc.sync.dma_start(xt_t[:], xdq[:], transpose=True)
            xt_qs.append(xt_t)

        ob = obp.tile([128, N2], bf16, tag="ob")
        for nq in range(NQ):
            pst = psp.tile([128, nq_width], f32, tag=f"ps{nq}")
            for kb in range(KB):
                nc.tensor.matmul(
                    pst[:],
                    xt_qs[kb // KBQ][:, kb % KBQ, :],
                    wt_t[:, kb, nq * nq_width:(nq + 1) * nq_width],
                    start=(kb == 0), stop=(kb == KB - 1))
            nc.scalar.copy(ob[:, nq * nq_width:(nq + 1) * nq_width], pst[:])
        nc.sync.dma_start(o_d[mt * BLK:(mt + 1) * BLK, :], ob[:])


def build_nc(m2, n2, k, **kw):
    nc = bacc.Bacc("TRN2", target_bir_lowering=False, debug=False, num_devices=NCORES)
    x_d = nc.dram_tensor("x", [m2, k], dt.bfloat16, kind="ExternalInput").ap()
    w_d = nc.dram_tensor("w", [n2, k], dt.float32, kind="ExternalInput").ap()
    ws_d = nc.dram_tensor("ws", [n2 // BLK, k // BLK], dt.float32, kind="ExternalInput").ap()
    o_d = nc.dram_tensor("o", [m2, n2], dt.bfloat16, kind="ExternalOutput").ap()
    with tile.TileContext(nc) as tc, ExitStack() as ctx:
        emit_kernel(ctx, tc, o_d, x_d, w_d, ws_d, **kw)
    nc.compile()
    return nc


_cache = {}


def _get_nc():
    if "nc" not in _cache:
        _cache["nc"] = build_nc(M // MSH, N // NSH, K)
    return _cache["nc"]


def kernel(input, weight_fp8, weight_scale, _trace=False, _trace_kwargs=None):
    input = np.asarray(input)
    if input.dtype != ml_dtypes.bfloat16:
        input = input.astype(ml_dtypes.bfloat16)
    weight_fp8 = np.asarray(weight_fp8, dtype=np.float32)
    weight_scale = np.asarray(weight_scale, dtype=np.float32)
    M2, N2 = M // MSH, N // NSH
    NSB = N2 // BLK

    in_maps = []
    for c in range(NCORES):
        mi, ni = divmod(c, NSH)
        in_maps.append({
            "x": np.ascontiguousarray(input[mi * M2:(mi + 1) * M2]),
            "w": np.ascontiguousarray(weight_fp8[ni * N2:(ni + 1) * N2]),
            "ws": np.ascontiguousarray(weight_scale[ni * NSB:(ni + 1) * NSB]),
        })

    nc = _get_nc()
    kw = {}
    if _trace:
        kw = dict(trace=True, **(_trace_kwargs or {}))
    res = run_bass_kernel_spmd(nc, in_maps, core_ids=list(range(NCORES)), **kw)

    out = np.empty((M, N), dtype=ml_dtypes.bfloat16)
    for c in range(NCORES):
        mi, ni = divmod(c, NSH)
        out[mi * M2:(mi + 1) * M2, ni * N2:(ni + 1) * N2] = res.results[c]["o"]
    if _trace:
        return out, res
    return out

